# revision 40
# baseline (speedup 1.0000x reference)
"""Multi-head causal self-attention (B=2, S=4096, D=512, H=8) on 8 trn2 cores.

Sharding: batch*heads = 16 (b,h) pairs -> 2 heads per core (head-parallel,
qkv weight columns sharded per head group). Zero cross-core communication.

Schedule (per-j-tile software pipeline, paced by the ScalarE exp stream):
  - Scores computed transposed (keys on partitions): per j-tile one sc psum
    tile [128, 1024] holds both heads; ONE exp ACTIVATE covers both heads
    (diag tiles use 3D APs to skip the masked query range). Causal mask via
    static mask tiles (DVE mul). Scores for tile j+1 are emitted ahead of
    tile j's AV so the ACT stream never waits on the PE.
  - AV accumulated per head in [65, 512] f32 psum over j-tiles; the
    appended ones column of V yields the softmax denominator for free.
  - Q/K/V^T projected just-in-time per 512-column block as chunk-granular
    aux tasks (~390ns of PE each) spread over the j-slots so no slot's PE
    work outruns the ~1.15us ACT pace; V for range t's own keys runs in
    range t's early slots (only needed at the diagonal), q/k for t+1 by
    slot njt-1.  Diag tiles processed in reverse (k=3 first, t>0) so each
    range ends on the full-length k=0 exp, covering the next range's
    first-scores emission.
  - O^T copied to SBUF in bf16 at range end; transpose/normalize/store
    split into per-head tail tasks on a global deferral queue popped by
    slots with no projection task (pops blocked while a psum accumulation
    group spans the slot - the 2-buffer aux pool would deadlock).
    Final range: per-block tails issue right after the diag AV that
    finalizes each 128-query block, shrinking the end drain.
  - Input xt delivered host-pre-blocked as [NI*P, KC*IT] with SBUF layout
    [P, NI, KC, IT] so each per-block DMA is one contiguous 4KB element
    per partition (the old [P, KC, S] layout split every transfer into
    512B descriptors and the input stream became descriptor-bound,
    landing block 1 at ~31us and starving the early ranges). Blocks go
    on separate hardware queues (a fused DMA single-queues at ~28us).
  - Prologue: w/xt0 split per chunk so the first projection's inputs land
    earliest; six bf16 PE heater matmuls bridge to input-land so a full
    free-running 4096-cycle HAM window sees continuous activity (fp32
    heaters do not register; a broken window leaves the clock gate at
    1.2GHz until ~17us); a dummy exp preloads the ACT exp table.
"""

import os
import sys

import numpy as np

for _p in ("/opt/trn_rl_repo", "/root/.axon_site/_ro/trn_rl_repo"):
    if os.path.isdir(_p) and _p not in sys.path:
        sys.path.append(_p)

import concourse.bass as bass
import concourse.tile as tile
from concourse import mybir
from concourse.masks import make_identity

F32 = mybir.dt.float32
BF16 = mybir.dt.bfloat16

B, S, D, H = 2, 4096, 512, 8
HD = 64          # head dim
NHC = 2          # heads per core
P = 128          # partitions
KC = D // P      # 4 contraction chunks for the projection
IT = 512         # query-range width
NI = S // IT     # 8 query ranges
JT = 128         # key-tile width
NJ = S // JT     # 32 key tiles
SCALE = 1.0 / np.sqrt(HD)  # 0.125
PW = 2 * NHC * IT           # pair-tile width: 2 j-tiles x 2 heads x 512


def build_nc():
    nc = bass.Bass()
    xt = nc.declare_dram_parameter("xt", [NI * P, KC * IT], BF16,
                                   isOutput=False)
    w = nc.declare_dram_parameter("w", [D, 3 * P], BF16, isOutput=False)
    out = nc.declare_dram_parameter("out", [S, NHC * HD], F32, isOutput=True)

    with tile.TileContext(nc) as tc:
        with (
            tc.tile_pool(name="singles", bufs=1) as singles,
            tc.tile_pool(name="epool", bufs=6) as epool,
            tc.tile_pool(name="otpool", bufs=8) as otpool,
            tc.tile_pool(name="vtpool", bufs=2) as vtpool,
            tc.tile_pool(name="outtp", bufs=6) as outtp,
            tc.tile_pool(name="rcpool", bufs=6) as rcpool,
            tc.tile_pool(name="ps_sc", bufs=2, space="PSUM") as ps_sc,
            tc.tile_pool(name="ps_av", bufs=2, space="PSUM") as ps_av,
            tc.tile_pool(name="ps_aux", bufs=2, space="PSUM") as ps_aux,
        ):
            # ---- resident tensors -------------------------------------
            xt_sb = singles.tile([P, NI, KC, IT], BF16, name="xt_sb")
            w_sb = singles.tile([P, KC, 3 * P], BF16, name="w_sb")
            qt = singles.tile([P, S], BF16, name="qt")
            kt = singles.tile([P, S], BF16, name="kt")
            # V per j-tile: [jt, 0:64] head0, [jt, 64] ones, [jt, 80:144]
            # head1, [jt, 144] ones. Head offsets are 32B-aligned.
            v_sb = singles.tile([P, NJ, 160], BF16, name="v_sb")
            masks_f = singles.tile([P, 4, IT], F32, name="masks_f")
            masks = singles.tile([P, 4, IT], BF16, name="masks")
            ident = singles.tile([P, P], F32, name="ident")
            ident_b = singles.tile([P, P], BF16, name="ident_b")
            heat_src = singles.tile([P, IT], F32, name="heat_src")
            ones_col = singles.tile([1, P], BF16, name="ones_col")
            dummy = singles.tile([P, 1], F32, name="dummy")

            # ---- input DMAs (blocked so block 0 lands early) ----------
            # ordered by need: xt block 0 first half, w_qk (gate the
            # first projections), xt block 0 second half, w_v, then the
            # remaining xt blocks fused into one descriptor set.
            # xt_sb block slice is contiguous per partition (4KB elems).
            w_r = w[:, :].rearrange("(c p) n -> p c n", p=P)
            xt_r = xt[:, :].rearrange("(b p) (c i) -> p b c i", p=P, c=KC)
            nc.sync.dma_start(out=w_sb[:, :, 0:P], in_=w_r[:, :, 0:P])
            nc.sync.dma_start(out=xt_sb[:, 0, 0, :], in_=xt_r[:, 0, 0, :])
            nc.sync.dma_start(out=xt_sb[:, 0, 1, :], in_=xt_r[:, 0, 1, :])
            nc.sync.dma_start(out=w_sb[:, :, P : 2 * P], in_=w_r[:, :, P : 2 * P])
            nc.sync.dma_start(out=xt_sb[:, 0, 2, :], in_=xt_r[:, 0, 2, :])
            nc.sync.dma_start(out=xt_sb[:, 0, 3, :], in_=xt_r[:, 0, 3, :])
            nc.sync.dma_start(out=w_sb[:, :, 2 * P :], in_=w_r[:, :, 2 * P :])
            # blocks 1..7 as separate DMAs: each gets its own hardware
            # queue so they stream in parallel (a fused single-queue DMA
            # takes ~28us and head-of-line-blocks the early projections)
            for blk in range(1, NI):
                nc.sync.dma_start(out=xt_sb[:, blk, :, :],
                                  in_=xt_r[:, blk, :, :])

            # ---- constants; PE heater + ACT table preload during DMA --
            nc.vector.memset(heat_src, 0.5)
            exp_f = mybir.ActivationFunctionType.Exp
            nc.scalar.activation(dummy, heat_src[:, 0:1], exp_f)
            nc.vector.memset(ones_col, 1.0)
            # heater: bf16 matmuls (fp32 LOW_HIGH ones do not register as
            # HAM activity) sized to end about when xt block 0 lands, so
            # the clock gate goes warm for the first projections
            heat_b = singles.tile([P, IT], BF16, name="heat_b")
            nc.vector.tensor_copy(heat_b, heat_src)
            # bridge PE activity to when xt block 0 lands (~12us) so
            # the HAM busy-window spans heater+projections with no gap
            for _ in range(6):
                hp = ps_aux.tile([P, IT], F32, tag="aux", name="hp")
                nc.tensor.matmul(
                    hp,
                    lhsT=heat_b[:, 0:P],
                    rhs=heat_b,
                    start=True,
                    stop=True,
                )
            make_identity(nc, ident)
            nc.vector.tensor_copy(ident_b, ident)
            nc.vector.memset(v_sb[:, :, 64:65], 1.0)
            nc.vector.memset(v_sb[:, :, 144:145], 1.0)
            # mask k: keep (=1) iff x - p - 128k >= 0, else 0
            for k in range(4):
                nc.gpsimd.memset(masks_f[:, k, :], 1.0)
                nc.gpsimd.affine_select(
                    out=masks_f[:, k, :],
                    in_=masks_f[:, k, :],
                    compare_op=mybir.AluOpType.is_ge,
                    fill=0.0,
                    base=-JT * k,
                    pattern=[[1, IT]],
                    channel_multiplier=-1,
                )
            nc.vector.tensor_copy(masks, masks_f)

            # ---- just-in-time projections per 512-column block --------
            def proj_finish(r, which, ps):
                sl = slice(r * IT, (r + 1) * IT)
                if which == 0:
                    nc.vector.tensor_copy(qt[:, sl], ps)
                elif which == 1:
                    if r == 0:
                        # split so scores(0,0) (keys 0:128) can issue
                        # before the full cast completes
                        nc.vector.tensor_copy(kt[:, 0:JT], ps[:, 0:JT])
                        nc.vector.tensor_copy(kt[:, JT:IT], ps[:, JT:IT])
                    else:
                        nc.vector.tensor_copy(kt[:, sl], ps)
                else:
                    vt = vtpool.tile([P, IT], BF16, tag="vt", name="vt")
                    nc.vector.tensor_copy(vt, ps)
                    return vt
                return None

            def proj(r, which):
                """which: 0=Q -> qt, 1=K -> kt, 2=V^T -> vt (full, for the
                prologue)."""
                ps = ps_aux.tile([P, IT], F32, tag="aux", name="ps_p")
                for c in range(KC):
                    nc.tensor.matmul(
                        ps,
                        lhsT=w_sb[:, c, which * P : (which + 1) * P],
                        rhs=xt_sb[:, r, c, :],
                        start=(c == 0),
                        stop=(c == KC - 1),
                    )
                return proj_finish(r, which, ps)

            def vtr(r, jj, st):
                """V^T [dim, key] -> v_sb [key, dim] for key-tile 4r+jj
                via PE transpose."""
                vt = st["vt"]
                ksl = slice(jj * JT, (jj + 1) * JT)
                tv = ps_aux.tile([P, P], BF16, tag="aux", name="tv")
                nc.tensor.transpose(tv, vt[:, ksl], ident_b)
                nc.vector.tensor_copy(v_sb[:, 4 * r + jj, 0:HD], tv[:, 0:HD])
                nc.vector.tensor_copy(
                    v_sb[:, 4 * r + jj, 80 : 80 + HD], tv[:, HD:P]
                )

            def proj_tasks(r, which):
                """Chunk-granular projection tasks (~390ns of PE each) so
                interleaving them into j-slots never outruns the ACT
                pace; for V the 4 per-key-tile transposes are separate
                tasks."""
                st = {}

                def chunk(c):
                    if c == 0:
                        st["ps"] = ps_aux.tile([P, IT], F32, tag="aux",
                                               name="ps_p")
                    nc.tensor.matmul(
                        st["ps"],
                        lhsT=w_sb[:, c, which * P : (which + 1) * P],
                        rhs=xt_sb[:, r, c, :],
                        start=(c == 0),
                        stop=(c == KC - 1),
                    )
                    if c == KC - 1:
                        st["vt"] = proj_finish(r, which, st["ps"])

                # tags: 'o'/'c' mark the open/close of the multi-slot psum
                # accumulation group (a deferred tail task popping between
                # them would rotate the 2-buffer aux pool onto the open
                # tile and deadlock)
                tasks = [(chunk, (c,),
                          "o" if c == 0 else ("c" if c == KC - 1 else "m"))
                         for c in range(KC)]
                if which == 2:
                    tasks += [(vtr, (r, jj, st), "n") for jj in range(4)]
                return tasks

            # ---- attention pieces (pair granularity) ------------------
            def kofs(t, j):
                njt = 4 * (t + 1)
                k = j - (njt - 4)
                return k, (JT * k if k > 0 else 0)

            def emit_scores(t, j):
                """sc tile [128, 1024]: h0 queries in 0:512, h1 in
                512:1024; diag tiles only compute queries >= offs."""
                k, offs = kofs(t, j)
                i0 = t * IT
                sc = ps_sc.tile([P, 2 * IT], F32, tag="sc", name="sc")
                for h in range(NHC):
                    hsl = slice(HD * h, HD * (h + 1))
                    nc.tensor.matmul(
                        sc[:, h * IT + offs : (h + 1) * IT],
                        lhsT=kt[hsl, j * JT : (j + 1) * JT],
                        rhs=qt[hsl, i0 + offs : i0 + IT],
                        start=True,
                        stop=True,
                        tile_position=(HD * h, 0),
                    )
                return sc

            def emit_exp(t, j, sc):
                k, offs = kofs(t, j)
                e = epool.tile([P, 2 * IT], BF16, tag="e", name="e")
                if k <= 0:
                    nc.scalar.activation(e, sc, exp_f, scale=SCALE)
                else:
                    # one ACTIVATE covers both heads' valid query ranges
                    # via a 3D access pattern (saves ~260ns/instr overhead)
                    sc3 = sc.rearrange("p (h i) -> p h i", h=2)[:, :, offs:IT]
                    e3 = e.rearrange("p (h i) -> p h i", h=2)[:, :, offs:IT]
                    nc.scalar.activation(e3, sc3, exp_f, scale=SCALE)
                if k >= 0:
                    for h in range(NHC):
                        usl = slice(h * IT + offs, (h + 1) * IT)
                        nc.vector.tensor_mul(
                            e[:, usl], e[:, usl], masks[:, k, offs:IT]
                        )
                return e

            def emit_av(t, j, av, e, first, last):
                k, offs = kofs(t, j)
                for h in range(NHC):
                    nc.tensor.matmul(
                        av[h][:, offs:IT],
                        lhsT=v_sb[:, j, 80 * h : 80 * h + 65],
                        rhs=e[:, h * IT + offs : (h + 1) * IT],
                        start=first,
                        stop=last,
                    )

            def jorder(t):
                """j-tile processing order for range t: diagonal tiles
                reversed (k=3 first) so the range ends on the full-length
                k=0 exp, which covers the PE's emission of the next
                range's first scores (the short diag exps otherwise let
                ACT outrun the PE at every range handoff). The final
                range keeps ascending order: its per-block tails need
                block b final right after diag k=b."""
                njt = 4 * (t + 1)
                if t == 0 or t == NI - 1:
                    # t=0: the first processed tile must be the full
                    # k=0 (start=True only clears the columns it
                    # writes); final range: tails need ascending k
                    return list(range(njt))
                return list(range(njt - 4)) + [njt - 1, njt - 2,
                                               njt - 3, njt - 4]

            def tail_half(ot_tiles, ti0, blk, h, st, rng=None):
                """transpose one head of an O.T block back to natural
                layout, normalize by the denominator column. rng=None
                (final range): per-block tile, DMA on the last head.
                rng set: write into the shared per-range buffer; the
                last (blk, h) issues one DMA for the whole range (one
                queue+semaphore instead of four)."""
                if rng is None:
                    if h == 0:
                        st["out_t"] = outtp.tile([P, NHC * HD], F32,
                                                 tag="outt", name="out_t")
                    out_t = st["out_t"]
                else:
                    out_t = rng[:, blk, :]
                tr = ps_aux.tile([P, 65], BF16, tag="aux", name="tr")
                nc.tensor.transpose(
                    tr, ot_tiles[h][:, blk * P : (blk + 1) * P],
                    ident_b[0:65, 0:65],
                )
                rc = rcpool.tile([P, 1], F32, tag="rc", name="rc")
                nc.vector.reciprocal(rc, tr[:, 64:65])
                nc.vector.tensor_scalar_mul(
                    out_t[:, h * HD : (h + 1) * HD], tr[:, 0:64], rc
                )
                if h == NHC - 1 and rng is None:
                    r0 = ti0 + blk * P
                    nc.sync.dma_start(out=out[r0 : r0 + P, :], in_=out_t)
                elif h == NHC - 1 and blk == 3:
                    nc.sync.dma_start(
                        out=out[ti0 : ti0 + IT, :].rearrange(
                            "(m p) n -> p m n", p=P
                        ),
                        in_=rng,
                    )

            def tail_tasks(ot_tiles, ti0):
                rng = outtp.tile([P, 4, NHC * HD], F32, tag="outt",
                                 name="ob")
                tasks = []
                for blk in range(4):
                    for h in range(NHC):
                        tasks.append(
                            (tail_half, (ot_tiles, ti0, blk, h, {}, rng))
                        )
                return tasks

            # ---- main pipeline ----------------------------------------
            # prologue: range-0 projections (q/k first so the first
            # scores+exp issue as soon as block 0 lands), then V for
            # range 0's own keys (needed at j=0, range 0 is all-diag)
            proj(0, 0)
            proj(0, 1)
            sc_next = emit_scores(0, 0)
            vt0 = proj(0, 2)
            for jj in range(4):
                vtr(0, jj, {"vt": vt0})

            def place(slots, tasks, lo, hi):
                """spread tasks uniformly over slot window [lo, hi]"""
                n = len(tasks)
                w = hi - lo + 1
                for i, task in enumerate(tasks):
                    slots[lo + i * w // n].append(task)

            # tail-block tasks are deferrable (otpool bufs=8 keeps four
            # ranges of O.T alive): they queue globally and fill slots
            # that carry no projection task, so the over-subscribed early
            # ranges shed work into the later ranges' slack
            tail_q = []               # (origin_range, task)

            for t in range(NI):
                njt = 4 * (t + 1)
                av = [
                    ps_av.tile([65, IT], F32, tag="av", name=f"av{h}")
                    for h in range(NHC)
                ]
                # chunk-granular projection tasks spread over this
                # range's slots (sequential groups - interleaving would
                # deadlock the 2-buffer aux pool):
                #  - V for range t's own keys: needed at the diagonal
                #    (slot njt-4)  [range 0's V was done in the prologue]
                #  - q/k for range t+1: needed by scores(t+1,0) at slot
                #    njt-1
                slots = [[] for _ in range(njt)]
                ptasks = []
                if t > 0:
                    ptasks += proj_tasks(t, 2)
                if t + 1 < NI:
                    ptasks += proj_tasks(t + 1, 0) + proj_tasks(t + 1, 1)
                if ptasks:
                    place(slots, ptasks,
                          0, min(njt - 1, max(njt - 3, len(ptasks) // 4)))
                # a slot may absorb a deferred tail task only when no
                # psum accumulation group spans the pop point (pops run
                # after the slot's own tasks)
                can_pop = []
                open_cnt = 0
                for j in range(njt):
                    for _fn, _args, tag in slots[j]:
                        if tag == "o":
                            open_cnt += 1
                        elif tag == "c":
                            open_cnt -= 1
                    can_pop.append(open_cnt == 0)
                final_ot = None
                if t == NI - 1:
                    final_ot = [
                        otpool.tile([65, IT], BF16, tag="ot", name=f"otf{h}")
                        for h in range(NHC)
                    ]
                jo = jorder(t)
                for idx, j in enumerate(jo):
                    sc = sc_next
                    e = emit_exp(t, j, sc)
                    # next j-tile's scores ahead of this tile's AV so the
                    # ACT stream never waits on the PE
                    if idx + 1 < njt:
                        sc_next = emit_scores(t, jo[idx + 1])
                    elif t + 1 < NI:
                        sc_next = emit_scores(t + 1, jorder(t + 1)[0])
                    else:
                        sc_next = None
                    for fn, args, _tag in slots[idx]:
                        fn(*args)
                    npop = (2 if t == NI - 1 else 1) if can_pop[idx] else 0
                    for _ in range(npop):
                        if tail_q:
                            fn, args = tail_q.pop(0)[1]
                            fn(*args)
                    emit_av(t, j, av, e, idx == 0, idx == njt - 1)
                    # final range: block b is complete right after diag
                    # k=b's AV; tail it immediately so the end drain
                    # overlaps the remaining diag exp/AV work
                    if final_ot is not None:
                        k, _ = kofs(t, j)
                        if k >= 0:
                            bsl = slice(k * P, (k + 1) * P)
                            for h in range(NHC):
                                nc.vector.tensor_copy(
                                    final_ot[h][:, bsl], av[h][:, bsl]
                                )
                            st = {}
                            for h in range(NHC):
                                tail_half(final_ot, t * IT, k, h, st)
                # force-drain tail tasks whose O.T tiles are needed soon
                # (evac of range t+1 reuses the buffers of range t-3)
                while tail_q and tail_q[0][0] <= t - 3:
                    fn, args = tail_q.pop(0)[1]
                    fn(*args)
                if t + 1 < NI:
                    # range end: O.T out of PSUM as bf16 (frees av for the
                    # next range); transpose/normalize deferred into later
                    # ranges' spare slots
                    ot = []
                    for h in range(NHC):
                        o = otpool.tile([65, IT], BF16, tag="ot",
                                        name=f"ot{h}")
                        nc.vector.tensor_copy(o, av[h])
                        ot.append(o)
                    for task in tail_tasks(ot, t * IT):
                        tail_q.append((t, task))
                else:
                    # flush any tails still queued (off the critical
                    # output path - their stores issued per block above)
                    while tail_q:
                        fn, args = tail_q.pop(0)[1]
                        fn(*args)
    return nc


def legalize_waits(nc):
    """This toolchain's walrus allows at most ONE sync-wait per instruction;
    split extra waits onto preceding same-engine NoOps (same trick Tile uses
    for its own wait/update carriers)."""
    nsplit = 0
    for f in nc.m.functions:
        for blk in f.blocks:
            new_insts = []
            for inst in blk.instructions:
                si = getattr(inst, "sync_info", None)
                ow = list(si.on_wait) if (si is not None and si.on_wait) else []
                if len(ow) > 1:
                    for w_i, wcond in enumerate(ow[:-1]):
                        nsplit += 1
                        nop = mybir.InstNoOp(
                            name=f"{inst.name}-wsplit{w_i}",
                            sync_info=mybir.SyncInfo(on_wait=[wcond], on_update=[]),
                            bass_nofuse=True,
                            engine=inst.engine,
                        )
                        new_insts.append(nop)
                    si.on_wait = ow[-1:]
                new_insts.append(inst)
            try:
                blk.instructions[:] = new_insts
            except TypeError:
                blk.instructions = new_insts
    return nsplit


_NC_CACHE = None


def _get_nc():
    global _NC_CACHE
    if _NC_CACHE is None:
        nc = build_nc()
        legalize_waits(nc)
        _NC_CACHE = nc
    return _NC_CACHE


def shard_inputs(inputs, qkv_weights):
    import ml_dtypes

    bf16 = ml_dtypes.bfloat16
    x = np.ascontiguousarray(np.asarray(inputs, dtype=np.float32))
    wf = np.ascontiguousarray(np.asarray(qkv_weights, dtype=np.float32))
    in_maps = []
    for c in range(8):
        b, g = divmod(c, 4)
        lo = g * P
        # [D, S] -> [NI*P, KC*IT]: block-major so each per-block DMA reads
        # contiguous rows (kernel reconstructs xt_sb[p, blk, c, i])
        xt_c = np.ascontiguousarray(
            x[b].T.reshape(KC, P, NI, IT)
            .transpose(2, 1, 0, 3)
            .reshape(NI * P, KC * IT)
        ).astype(bf16)
        w_c = np.ascontiguousarray(
            np.concatenate(
                [wf[:, q * D + lo : q * D + lo + P] for q in range(3)], axis=1
            )
        ).astype(bf16)
        in_maps.append({"xt": xt_c, "w": w_c})
    return in_maps


def gather_outputs(results):
    out = np.empty((B, S, D), dtype=np.float32)
    for c in range(8):
        b, g = divmod(c, 4)
        out[b, :, g * P : (g + 1) * P] = results[c]["out"]
    return out


def run(in_maps, **kwargs):
    from concourse.bass_utils import run_bass_kernel_spmd

    return run_bass_kernel_spmd(_get_nc(), in_maps, list(range(8)), **kwargs)


def kernel(**inputs):
    in_maps = shard_inputs(inputs["inputs"], inputs["qkv_weights"])
    res = run(in_maps)
    return gather_outputs(res.results)
